# revision 26
# baseline (speedup 1.0000x reference)
"""Trainium2 Bass kernel for the Kalman graphical-model message-passing problem.

reference math (B=64, D=8, M=4, S=50000):
    m1 = -Qinv @ (xs - F @ x_past)            (B, D, S)
    m2 = FtQinv @ (x_fut - F @ xs)            (B, D, S)
    m3 = HtRinv @ ys_t - (HtRinv @ H) @ xs    (B, D, S)
with x_past/x_fut edge-replicated 1-sample shifts of xs along S.

Reformulated as pure (tiny matrix) x (data) products with host-precomputed
weights:
    m1 = A1 @ xs + B1 @ x_past        A1 = -Qinv,        B1 = Qinv @ F
    m2 = A2 @ xs + B2 @ x_fut         A2 = -F'QinvF,     B2 = F' @ Qinv
    m3 = A3 @ xs + C3 @ ys_t          A3 = -(C3 @ H),    C3 = H' @ Rinv

Design (fp16 I/O, memory-roofline oriented; 8-way batch data parallel):
  Host converts xs -> fp16 padded with edge-replicated halo columns
  (B,D,S+2), ys -> transposed fp16 ys_t (B,M,S); the output comes back fp16.
  That halves HBM traffic vs f32 (well within the 2e-2 rel-err budget).  Per
  core (8 batches), ONE supertile per batch covers the whole signal:
  partitions = 16 groups x 8 states, each group a 3125-sample stripe with 2
  halo columns, so cur/past/fut are column offsets 1/0/2 of one [128, 3127]
  tile loaded by a single 128-descriptor DMA (~6.25 KB runs, striping evenly
  over all 16 DMA engines).  The transposed ys loads as [64, 3125]
  (partition = (group, m)), letting ONE matmul contract all 4 observation
  dims -> 6 fp16 matmuls per 512-column PSUM half instead of 9.  PSUM drains
  (f32 -> fp16) are split between the vector and scalar engines.  Stores go
  out in two column pieces per message on the Pool SWDGE queue -- a third
  DMA queue independent of the load (sync HWDGE) queue, which also keeps DMA
  off the Act engine (mixing Act HWDGE stores with Act drains crashes HW).
  Measured: ~105-115 us vs the ~80 us 360 GB/s chip roofline for the 28.8
  MB/core of traffic (361 us for the f32 9-matmul baseline).
"""

import os
from contextlib import ExitStack

import numpy as np

import concourse.bacc as bacc
import concourse.bass as bass
import concourse.mybir as mybir
import concourse.tile as tile
from concourse.bass_utils import run_bass_kernel_spmd

F16 = mybir.dt.float16
F32 = mybir.dt.float32

B, D, M, S = 64, 8, 4, 50000
N_CORES = 8
BC = B // N_CORES  # batches per core
NG = 16            # sample groups packed into the 128 partitions
MW = 512           # matmul free-dim / PSUM bank width


def _build_nc(bc=BC, s=S):
    variant = os.environ.get("KERNEL_VARIANT", "full")  # perf bisection only
    assert s % NG == 0, s
    stride = s // NG          # samples per group (stored width)
    cols = stride + 2         # + past/fut halo columns

    sp = s + 2  # xs arrives host-padded with edge-replicated halo columns
    nc = bacc.Bacc(trn_type="TRN2")
    xs = nc.dram_tensor("xs", [bc, D, sp], F16, kind="ExternalInput")
    yt = nc.dram_tensor("yt", [bc, M, s], F16, kind="ExternalInput")
    w = nc.dram_tensor("w_all", [128, 6 * 128], F16, kind="ExternalInput")
    m_all = nc.dram_tensor("m_all", [bc, D, 3, s], F16, kind="ExternalOutput")

    with tile.TileContext(nc) as tc, ExitStack() as ctx:
        singles = ctx.enter_context(tc.tile_pool(name="singles", bufs=1))
        nbx = int(os.environ.get("KERNEL_BUFS_X", "7"))
        nbo = int(os.environ.get("KERNEL_BUFS_O", "5"))
        xp = ctx.enter_context(tc.tile_pool(name="xp", bufs=nbx))
        yp = ctx.enter_context(tc.tile_pool(name="yp", bufs=nbx))
        op = ctx.enter_context(tc.tile_pool(name="op", bufs=nbo))
        pp = ctx.enter_context(tc.tile_pool(name="pp", bufs=2, space="PSUM"))

        # w rides the (otherwise idle-at-start) SWDGE queue so batch 0's x
        # load leads the sync HWDGE queue
        w_sb = singles.tile([128, 6 * 128], F16, tag="w")
        nc.gpsimd.dma_start(out=w_sb[:], in_=w[:, :])
        wr = w_sb[:]

        for b in range(bc):
            xoff = b * D * sp
            yoff = b * M * s
            ooff = b * D * 3 * s

            # --- xs supertile -----------------------------------------------
            # xs is host-padded: padded[t] = x[t-1] with edge replication, so
            # column c of group g = x[g*stride + c - 1] and the whole tile is
            # ONE 128-descriptor DMA (stripes evenly over all 16 DMA engines).
            x_t = xp.tile([128, cols], F16, tag="x")
            nc.sync.dma_start(
                out=x_t[:, 0:cols],
                in_=bass.AP(xs, xoff, [[stride, NG], [sp, D], [1, cols]]),
            )

            # --- ys supertile (transposed on host: partition = (g, m)) -----
            y_t = yp.tile([64, stride], F16, tag="y")
            nc.sync.dma_start(
                out=y_t[:, :],
                in_=bass.AP(yt, yoff, [[stride, NG], [s, M], [1, stride]]),
            )

            if variant == "loads":
                continue
            o_t = op.tile([128, 3 * stride], F16, tag="o", name=f"o_{b}")
            # Store-piece boundaries (each must land on a 512-col half edge):
            # mid-batch pieces overlap store traffic with remaining compute.
            # Batch 0 starts storing early (fills the warmup DMA drought);
            # the last batch drains finely so the post-compute tail store is
            # tiny (~40 KB instead of 0.83 MB).
            if stride > 4 * MW and os.environ.get("KERNEL_SPLIT", "1") == "1":
                if b == 0:
                    splits = [2 * MW, 4 * MW]
                elif b == bc - 1:
                    splits = [2 * MW, 4 * MW, 6 * MW]
                else:
                    splits = [4 * MW]
                splits = [c for c in splits if c < stride]
            else:
                splits = []
            bounds = [0] + splits + [stride]
            pieces = list(zip(bounds[:-1], bounds[1:]))
            # stores ride the Pool SWDGE queue: independent of the load
            # (sync HWDGE) queue, and keeps DMA off the Act engine, which
            # crashes when mixing HWDGE stores with activation drains.
            store_eng = {
                "mixdrain": nc.sync,
                "vecdrain": nc.scalar,
            }.get(variant, nc.gpsimd)

            # --- matmuls + PSUM drain, in 512-column halves ----------------
            nh = -(-stride // MW)
            for hi in range(nh):
                h0 = hi * MW
                hw_ = min(MW, stride - h0)
                ps = [
                    pp.tile([128, MW], F32, tag=f"p{i}", name=f"p{i}_{b}_{hi}")
                    for i in range(3)
                ]
                cur = x_t[:, 1 + h0 : 1 + h0 + hw_]
                past = x_t[:, h0 : h0 + hw_]
                fut = x_t[:, 2 + h0 : 2 + h0 + hw_]
                p0 = ps[0][:, 0:hw_]
                p1 = ps[1][:, 0:hw_]
                p2 = ps[2][:, 0:hw_]
                nc.tensor.matmul(p0, wr[:, 0:128], cur, start=True, stop=False)
                nc.tensor.matmul(p0, wr[:, 128:256], past, start=False, stop=True)
                nc.tensor.matmul(p1, wr[:, 256:384], cur, start=True, stop=False)
                nc.tensor.matmul(p1, wr[:, 384:512], fut, start=False, stop=True)
                nc.tensor.matmul(p2, wr[:, 512:640], cur, start=True, stop=False)
                nc.tensor.matmul(
                    p2, wr[0:64, 640:768], y_t[:, h0 : h0 + hw_], start=False, stop=True
                )
                if variant == "nostores":
                    continue
                # drains (f32 -> fp16), balanced across vector + scalar
                if variant == "vecdrain":
                    nc.vector.tensor_copy(out=o_t[:, h0 : h0 + hw_], in_=p0)
                    nc.vector.tensor_copy(
                        out=o_t[:, stride + h0 : stride + h0 + hw_], in_=p1
                    )
                    nc.vector.tensor_copy(
                        out=o_t[:, 2 * stride + h0 : 2 * stride + h0 + hw_], in_=p2
                    )
                else:
                    nc.vector.tensor_copy(out=o_t[:, h0 : h0 + hw_], in_=p0)
                    nc.scalar.copy(
                        out=o_t[:, stride + h0 : stride + h0 + hw_], in_=p1
                    )
                    if hi % 2 == 0:
                        nc.scalar.copy(
                            out=o_t[:, 2 * stride + h0 : 2 * stride + h0 + hw_],
                            in_=p2,
                        )
                    else:
                        nc.vector.tensor_copy(
                            out=o_t[:, 2 * stride + h0 : 2 * stride + h0 + hw_],
                            in_=p2,
                        )
                # mid-batch store pieces whose columns are fully drained
                for c0, c1 in pieces[:-1]:
                    if h0 + hw_ == c1:
                        for o in range(3):
                            store_eng.dma_start(
                                out=bass.AP(
                                    m_all,
                                    ooff + o * s + c0,
                                    [[stride, NG], [3 * s, D], [1, c1 - c0]],
                                ),
                                in_=o_t[:, o * stride + c0 : o * stride + c1],
                            )

            if variant == "nostores":
                continue
            # --- tail store: the final column piece ------------------------
            c0, c1 = pieces[-1]
            for o in range(3):
                store_eng.dma_start(
                    out=bass.AP(
                        m_all,
                        ooff + o * s + c0,
                        [[stride, NG], [3 * s, D], [1, c1 - c0]],
                    ),
                    in_=o_t[:, o * stride + c0 : o * stride + c1],
                )
    nc.finalize()
    return nc


def _build_weights(F, H, Q, R):
    """Host-side precompute (init-time work in the torch module)."""
    F64 = np.asarray(F, np.float64)
    H64 = np.asarray(H, np.float64)
    Q64 = np.asarray(Q, np.float64)
    R64 = np.asarray(R, np.float64)
    Qinv = np.linalg.inv(Q64)
    Rinv = np.linalg.inv(R64)
    A1 = -Qinv
    B1 = Qinv @ F64
    B2 = F64.T @ Qinv
    A2 = -(B2 @ F64)
    C3 = H64.T @ Rinv          # (D, M)
    A3 = -(C3 @ H64)

    w = np.zeros((128, 6 * 128), np.float64)
    eye = np.eye(NG)
    for i, A in enumerate([A1, B1, A2, B2, A3]):
        # lhsT[8g+j, 8g+i] = A[i, j]  ->  block-diag of A.T
        w[:, i * 128 : (i + 1) * 128] = np.kron(eye, A.T)
    for g in range(NG):
        # lhsT[4g+m, 8g+i] = C3[i, m]
        w[4 * g : 4 * g + 4, 640 + 8 * g : 640 + 8 * g + 8] = C3.T
    return w.astype(np.float16)


_CACHE = {}


def _get_nc(bc=BC, s=S):
    key = (bc, s)
    if key not in _CACHE:
        _CACHE[key] = _build_nc(bc, s)
    return _CACHE[key]


def run(xs, ys, F, H, Q, R, trace=False, bc=BC, s=S):
    """Shard across 8 cores, run, gather.  Returns ((m1, m2, m3), results)."""
    xs = np.asarray(xs)
    nb = xs.shape[0]
    assert nb == bc * N_CORES and xs.shape[1:] == (D, s), xs.shape
    # pad the sample axis with edge-replicated halo columns (see _build_nc)
    xs16 = np.empty((nb, D, s + 2), np.float16)
    xs16[:, :, 1 : s + 1] = xs
    xs16[:, :, 0] = xs16[:, :, 1]
    xs16[:, :, s + 1] = xs16[:, :, s]
    yt16 = np.asarray(ys, np.float32).transpose(0, 2, 1).astype(np.float16)
    w_all = _build_weights(F, H, Q, R)
    assert yt16.shape == (nb, M, s), yt16.shape

    nc = _get_nc(bc, s)
    in_maps = [
        {
            "xs": xs16[i * bc : (i + 1) * bc],
            "yt": yt16[i * bc : (i + 1) * bc],
            "w_all": w_all,
        }
        for i in range(N_CORES)
    ]
    res = run_bass_kernel_spmd(nc, in_maps, core_ids=list(range(N_CORES)), trace=trace)
    m_full = np.concatenate([r["m_all"] for r in res.results], axis=0)  # (B,D,3,s)
    outs = tuple(m_full[:, :, i, :].astype(np.float32) for i in range(3))
    return outs, res


def kernel(xs, ys, F, H, Q, R):
    trace = bool(int(os.environ.get("KERNEL_TRACE", "0")))
    outs, _ = run(xs, ys, F, H, Q, R, trace=trace)
    return outs


# revision 27
# speedup vs baseline: 1.0033x; 1.0033x over previous
"""Trainium2 Bass kernel for the Kalman graphical-model message-passing problem.

reference math (B=64, D=8, M=4, S=50000):
    m1 = -Qinv @ (xs - F @ x_past)            (B, D, S)
    m2 = FtQinv @ (x_fut - F @ xs)            (B, D, S)
    m3 = HtRinv @ ys_t - (HtRinv @ H) @ xs    (B, D, S)
with x_past/x_fut edge-replicated 1-sample shifts of xs along S.

Reformulated as pure (tiny matrix) x (data) products with host-precomputed
weights:
    m1 = A1 @ xs + B1 @ x_past        A1 = -Qinv,        B1 = Qinv @ F
    m2 = A2 @ xs + B2 @ x_fut         A2 = -F'QinvF,     B2 = F' @ Qinv
    m3 = A3 @ xs + C3 @ ys_t          A3 = -(C3 @ H),    C3 = H' @ Rinv

Design (fp16 I/O, memory-roofline oriented; 8-way batch data parallel):
  Host converts xs -> fp16 padded with edge-replicated halo columns
  (B,D,S+2), ys -> transposed fp16 ys_t (B,M,S); the output comes back fp16.
  That halves HBM traffic vs f32 (well within the 2e-2 rel-err budget).  Per
  core (8 batches), ONE supertile per batch covers the whole signal:
  partitions = 16 groups x 8 states, each group a 3125-sample stripe with 2
  halo columns, so cur/past/fut are column offsets 1/0/2 of one [128, 3127]
  tile loaded by a single 128-descriptor DMA (~6.25 KB runs, striping evenly
  over all 16 DMA engines).  The transposed ys loads as [64, 3125]
  (partition = (group, m)), letting ONE matmul contract all 4 observation
  dims -> 6 fp16 matmuls per 512-column PSUM half instead of 9.  PSUM drains
  (f32 -> fp16) are split between the vector and scalar engines.  Stores go
  out in two column pieces per message on the Pool SWDGE queue -- a third
  DMA queue independent of the load (sync HWDGE) queue, which also keeps DMA
  off the Act engine (mixing Act HWDGE stores with Act drains crashes HW).
  Measured: ~105-115 us vs the ~80 us 360 GB/s chip roofline for the 28.8
  MB/core of traffic (361 us for the f32 9-matmul baseline).
"""

import os
from contextlib import ExitStack

import numpy as np

import concourse.bacc as bacc
import concourse.bass as bass
import concourse.mybir as mybir
import concourse.tile as tile
from concourse.bass_utils import run_bass_kernel_spmd

F16 = mybir.dt.float16
F32 = mybir.dt.float32

B, D, M, S = 64, 8, 4, 50000
N_CORES = 8
BC = B // N_CORES  # batches per core
NG = 16            # sample groups packed into the 128 partitions
MW = 512           # matmul free-dim / PSUM bank width


def _build_nc(bc=BC, s=S):
    variant = os.environ.get("KERNEL_VARIANT", "full")  # perf bisection only
    assert s % NG == 0, s
    stride = s // NG          # samples per group (stored width)
    cols = stride + 2         # + past/fut halo columns

    sp = s + 2  # xs arrives host-padded with edge-replicated halo columns
    nc = bacc.Bacc(trn_type="TRN2")
    xs = nc.dram_tensor("xs", [bc, D, sp], F16, kind="ExternalInput")
    yt = nc.dram_tensor("yt", [bc, M, s], F16, kind="ExternalInput")
    w = nc.dram_tensor("w_all", [128, 6 * 128], F16, kind="ExternalInput")
    m_all = nc.dram_tensor("m_all", [bc, D, 3, s], F16, kind="ExternalOutput")

    with tile.TileContext(nc) as tc, ExitStack() as ctx:
        singles = ctx.enter_context(tc.tile_pool(name="singles", bufs=1))
        nbx = int(os.environ.get("KERNEL_BUFS_X", "8"))
        nbo = int(os.environ.get("KERNEL_BUFS_O", "5"))
        xp = ctx.enter_context(tc.tile_pool(name="xp", bufs=nbx))
        yp = ctx.enter_context(tc.tile_pool(name="yp", bufs=nbx))
        op = ctx.enter_context(tc.tile_pool(name="op", bufs=nbo))
        pp = ctx.enter_context(tc.tile_pool(name="pp", bufs=2, space="PSUM"))

        # w rides the (otherwise idle-at-start) SWDGE queue so batch 0's x
        # load leads the sync HWDGE queue
        w_sb = singles.tile([128, 6 * 128], F16, tag="w")
        nc.gpsimd.dma_start(out=w_sb[:], in_=w[:, :])
        wr = w_sb[:]

        for b in range(bc):
            xoff = b * D * sp
            yoff = b * M * s
            ooff = b * D * 3 * s

            # --- xs supertile -----------------------------------------------
            # xs is host-padded: padded[t] = x[t-1] with edge replication, so
            # column c of group g = x[g*stride + c - 1] and the whole tile is
            # ONE 128-descriptor DMA (stripes evenly over all 16 DMA engines).
            x_t = xp.tile([128, cols], F16, tag="x")
            nc.sync.dma_start(
                out=x_t[:, 0:cols],
                in_=bass.AP(xs, xoff, [[stride, NG], [sp, D], [1, cols]]),
            )

            # --- ys supertile (transposed on host: partition = (g, m)) -----
            y_t = yp.tile([64, stride], F16, tag="y")
            nc.sync.dma_start(
                out=y_t[:, :],
                in_=bass.AP(yt, yoff, [[stride, NG], [s, M], [1, stride]]),
            )

            if variant == "loads":
                continue
            o_t = op.tile([128, 3 * stride], F16, tag="o", name=f"o_{b}")
            # Store-piece boundaries (each must land on a 512-col half edge):
            # mid-batch pieces overlap store traffic with remaining compute.
            # Batch 0 starts storing early (fills the warmup DMA drought);
            # the last batch drains finely so the post-compute tail store is
            # tiny (~40 KB instead of 0.83 MB).
            if stride > 4 * MW and os.environ.get("KERNEL_SPLIT", "1") == "1":
                if b == 0:
                    splits = [2 * MW, 4 * MW]
                elif b == bc - 1:
                    splits = [2 * MW, 4 * MW, 6 * MW]
                else:
                    splits = [4 * MW]
                splits = [c for c in splits if c < stride]
            else:
                splits = []
            bounds = [0] + splits + [stride]
            pieces = list(zip(bounds[:-1], bounds[1:]))
            # stores ride the Pool SWDGE queue: independent of the load
            # (sync HWDGE) queue, and keeps DMA off the Act engine, which
            # crashes when mixing HWDGE stores with activation drains.
            store_eng = {
                "mixdrain": nc.sync,
                "vecdrain": nc.scalar,
            }.get(variant, nc.gpsimd)

            # --- matmuls + PSUM drain, in 512-column halves ----------------
            nh = -(-stride // MW)
            for hi in range(nh):
                h0 = hi * MW
                hw_ = min(MW, stride - h0)
                ps = [
                    pp.tile([128, MW], F32, tag=f"p{i}", name=f"p{i}_{b}_{hi}")
                    for i in range(3)
                ]
                cur = x_t[:, 1 + h0 : 1 + h0 + hw_]
                past = x_t[:, h0 : h0 + hw_]
                fut = x_t[:, 2 + h0 : 2 + h0 + hw_]
                p0 = ps[0][:, 0:hw_]
                p1 = ps[1][:, 0:hw_]
                p2 = ps[2][:, 0:hw_]
                nc.tensor.matmul(p0, wr[:, 0:128], cur, start=True, stop=False)
                nc.tensor.matmul(p0, wr[:, 128:256], past, start=False, stop=True)
                nc.tensor.matmul(p1, wr[:, 256:384], cur, start=True, stop=False)
                nc.tensor.matmul(p1, wr[:, 384:512], fut, start=False, stop=True)
                nc.tensor.matmul(p2, wr[:, 512:640], cur, start=True, stop=False)
                nc.tensor.matmul(
                    p2, wr[0:64, 640:768], y_t[:, h0 : h0 + hw_], start=False, stop=True
                )
                if variant == "nostores":
                    continue
                # drains (f32 -> fp16), balanced across vector + scalar
                if variant == "vecdrain":
                    nc.vector.tensor_copy(out=o_t[:, h0 : h0 + hw_], in_=p0)
                    nc.vector.tensor_copy(
                        out=o_t[:, stride + h0 : stride + h0 + hw_], in_=p1
                    )
                    nc.vector.tensor_copy(
                        out=o_t[:, 2 * stride + h0 : 2 * stride + h0 + hw_], in_=p2
                    )
                else:
                    nc.vector.tensor_copy(out=o_t[:, h0 : h0 + hw_], in_=p0)
                    nc.scalar.copy(
                        out=o_t[:, stride + h0 : stride + h0 + hw_], in_=p1
                    )
                    if hi % 2 == 0:
                        nc.scalar.copy(
                            out=o_t[:, 2 * stride + h0 : 2 * stride + h0 + hw_],
                            in_=p2,
                        )
                    else:
                        nc.vector.tensor_copy(
                            out=o_t[:, 2 * stride + h0 : 2 * stride + h0 + hw_],
                            in_=p2,
                        )
                # mid-batch store pieces whose columns are fully drained
                for c0, c1 in pieces[:-1]:
                    if h0 + hw_ == c1:
                        for o in range(3):
                            store_eng.dma_start(
                                out=bass.AP(
                                    m_all,
                                    ooff + o * s + c0,
                                    [[stride, NG], [3 * s, D], [1, c1 - c0]],
                                ),
                                in_=o_t[:, o * stride + c0 : o * stride + c1],
                            )

            if variant == "nostores":
                continue
            # --- tail store: the final column piece ------------------------
            c0, c1 = pieces[-1]
            for o in range(3):
                store_eng.dma_start(
                    out=bass.AP(
                        m_all,
                        ooff + o * s + c0,
                        [[stride, NG], [3 * s, D], [1, c1 - c0]],
                    ),
                    in_=o_t[:, o * stride + c0 : o * stride + c1],
                )
    nc.finalize()
    return nc


def _build_weights(F, H, Q, R):
    """Host-side precompute (init-time work in the torch module)."""
    F64 = np.asarray(F, np.float64)
    H64 = np.asarray(H, np.float64)
    Q64 = np.asarray(Q, np.float64)
    R64 = np.asarray(R, np.float64)
    Qinv = np.linalg.inv(Q64)
    Rinv = np.linalg.inv(R64)
    A1 = -Qinv
    B1 = Qinv @ F64
    B2 = F64.T @ Qinv
    A2 = -(B2 @ F64)
    C3 = H64.T @ Rinv          # (D, M)
    A3 = -(C3 @ H64)

    w = np.zeros((128, 6 * 128), np.float64)
    eye = np.eye(NG)
    for i, A in enumerate([A1, B1, A2, B2, A3]):
        # lhsT[8g+j, 8g+i] = A[i, j]  ->  block-diag of A.T
        w[:, i * 128 : (i + 1) * 128] = np.kron(eye, A.T)
    for g in range(NG):
        # lhsT[4g+m, 8g+i] = C3[i, m]
        w[4 * g : 4 * g + 4, 640 + 8 * g : 640 + 8 * g + 8] = C3.T
    return w.astype(np.float16)


_CACHE = {}


def _get_nc(bc=BC, s=S):
    key = (bc, s)
    if key not in _CACHE:
        _CACHE[key] = _build_nc(bc, s)
    return _CACHE[key]


def run(xs, ys, F, H, Q, R, trace=False, bc=BC, s=S):
    """Shard across 8 cores, run, gather.  Returns ((m1, m2, m3), results)."""
    xs = np.asarray(xs)
    nb = xs.shape[0]
    assert nb == bc * N_CORES and xs.shape[1:] == (D, s), xs.shape
    # pad the sample axis with edge-replicated halo columns (see _build_nc)
    xs16 = np.empty((nb, D, s + 2), np.float16)
    xs16[:, :, 1 : s + 1] = xs
    xs16[:, :, 0] = xs16[:, :, 1]
    xs16[:, :, s + 1] = xs16[:, :, s]
    yt16 = np.asarray(ys, np.float32).transpose(0, 2, 1).astype(np.float16)
    w_all = _build_weights(F, H, Q, R)
    assert yt16.shape == (nb, M, s), yt16.shape

    nc = _get_nc(bc, s)
    in_maps = [
        {
            "xs": xs16[i * bc : (i + 1) * bc],
            "yt": yt16[i * bc : (i + 1) * bc],
            "w_all": w_all,
        }
        for i in range(N_CORES)
    ]
    res = run_bass_kernel_spmd(nc, in_maps, core_ids=list(range(N_CORES)), trace=trace)
    m_full = np.concatenate([r["m_all"] for r in res.results], axis=0)  # (B,D,3,s)
    outs = tuple(m_full[:, :, i, :].astype(np.float32) for i in range(3))
    return outs, res


def kernel(xs, ys, F, H, Q, R):
    trace = bool(int(os.environ.get("KERNEL_TRACE", "0")))
    outs, _ = run(xs, ys, F, H, Q, R, trace=trace)
    return outs
